# revision 11
# baseline (speedup 1.0000x reference)
"""Trainium2 Bass kernel for the soft-logic-gate network (nn_Logic).

Math: for each output neuron o with wired inputs a[o], b[o]:
    av = x[:, a[o]], bv = x[:, b[o]], nw = softmax(weights[o])   (16 gates)
    r[:, o] = sum_g nw[g] * gate_g(av, bv)

Every one of the 16 soft gates is affine in (1, av, bv, av*bv), so the
mixture collapses to
    r = w0 + w1*av + w2*bv + w3*av*bv,   [w0..w3] = softmax(weights) @ C
with C a constant [16, 4] matrix. The kernel computes softmax+C on-chip,
gathers av/bv rows from a transposed fp16 x table with the GPSIMD dma_gather
instruction, and does the affine combine with 2 ACT + 2 DVE ops per
128-neuron chunk.

dma_gather takes int16 row indices, so the 65536-row table is treated as two
32768-row halves and neurons are grouped by which half (a, b) each fall in;
groups are padded to 128-row chunks and balanced round-robin across the 8
cores (out_dim sharding). Gathered row j lands at partition j%128,
slot j//128 — the whole layout (indices, weights, output rows) uses that
slot order, and the host undoes the permutation at the end.
"""

import numpy as np

import concourse.bacc as bacc
import concourse.mybir as mybir
import concourse.tile as tile
from concourse.bass_utils import run_bass_kernel_spmd

N_CORES = 8
IN_DIM = 65536
OUT_DIM = 65536
BATCH = 1024
NG = 16
P = 128
HALF = IN_DIM // 2          # dma_gather int16 index limit is 32767
SUBS = 4                    # slots (128-row groups) per dma_gather

# Gate g = c0 + c1*av + c2*bv + c3*av*bv, rows in module gate order 0..15.
GATE_COEF = np.array(
    [
        [0, 0, 0, 0],    # FALSE
        [0, 0, 0, 1],    # AND
        [0, 1, 0, -1],   # A AND NOT B
        [0, 1, 0, 0],    # A
        [0, 0, 1, -1],   # NOT A AND B
        [0, 0, 1, 0],    # B
        [0, 1, 1, -2],   # XOR
        [0, 1, 1, -1],   # OR
        [1, -1, -1, 1],  # NOR
        [1, -1, -1, 2],  # XNOR
        [1, 0, -1, 0],   # NOT B
        [1, 0, -1, 1],   # A OR NOT B
        [1, -1, 0, 0],   # NOT A
        [1, -1, 0, 1],   # NOT A OR B
        [1, 0, 0, -1],   # NAND
        [1, 0, 0, 0],    # TRUE
    ],
    dtype=np.float32,
)

f32 = mybir.dt.float32
f16 = mybir.dt.float16
i16 = mybir.dt.int16

# Gather-table dtype: float16 halves gather traffic (x is uniform [0,1), so
# fp16 quantization adds ~5e-4 relative error).
XT_DT = f16
XT_DT_NP = np.float16


def build_program(sections, in_dim=IN_DIM, batch=BATCH, half=None, bufs=3):
    """Trace the per-core Bass/Tile program (identical on all cores).

    sections: list of (ha, hb, n_chunks_g) — per neuron-group table halves
    and chunk counts (chunks of 128 neurons), shared by all cores.
    """
    if half is None:
        half = in_dim // 2
    n = sum(s for _, _, s in sections)    # total chunks (= slots/partition)

    nc = bacc.Bacc("TRN2", target_bir_lowering=False, debug=False,
                   enable_asserts=False, num_swdge_queues=2)

    xt = nc.dram_tensor("xt", [in_dim, batch], XT_DT, kind="ExternalInput")
    ia = nc.dram_tensor("ia", [P, n * 8], i16, kind="ExternalInput")
    ib = nc.dram_tensor("ib", [P, n * 8], i16, kind="ExternalInput")
    wg = nc.dram_tensor("wg", [P, n * NG], f32, kind="ExternalInput")
    out = nc.dram_tensor("out", [n * P, batch], f32, kind="ExternalOutput")

    # [128, 4*16] constant: row-replicated C^T (q-major, g-minor).
    cmat_np = np.tile(GATE_COEF.T.reshape(1, 4 * NG), (P, 1)).astype(np.float32)
    cm = nc.inline_tensor(cmat_np, name="cmat")

    with tile.TileContext(nc) as tc:
        with (
            tc.tile_pool(name="const", bufs=1) as cpool,
            tc.tile_pool(name="gath", bufs=bufs) as gpool,
            tc.tile_pool(name="work", bufs=2 * bufs) as wpool,
        ):
            # --- constant loads -------------------------------------------
            # indices: position i (slot order) lives at [i%16 (+16k), i//16].
            iasb = cpool.tile([P, n * 8], i16)
            ibsb = cpool.tile([P, n * 8], i16)
            nc.sync.dma_start(out=iasb[:], in_=ia[:])
            nc.sync.dma_start(out=ibsb[:], in_=ib[:])
            csb = cpool.tile([P, 4 * NG], f32)
            nc.sync.dma_start(out=csb[:], in_=cm.ap())
            wsb = cpool.tile([P, n * NG], f32)
            nc.sync.dma_start(out=wsb[:], in_=wg[:])

            # --- softmax(weights) @ C  ->  4 coefficient planes -----------
            # weights layout in SBUF: [p, (t, g)] with t = chunk slot
            # (neuron position = t*128 + p). randn weights are small, so exp
            # without max-subtraction is safe in f32.
            esb = cpool.tile([P, n * NG], f32)
            nc.scalar.activation(esb[:], wsb[:], mybir.ActivationFunctionType.Exp)
            ssb = cpool.tile([P, n], f32)
            nc.vector.reduce_sum(
                ssb[:], esb[:].rearrange("p (n g) -> p n g", g=NG),
                axis=mybir.AxisListType.X)
            rsb = cpool.tile([P, n], f32)
            nc.vector.reciprocal(rsb[:], ssb[:])

            wq = cpool.tile([P, 4 * n], f32)   # [p, (q, t)]
            tmp = cpool.tile([P, n * NG], f32)
            for q in range(4):
                cqb = csb[:, q * NG:(q + 1) * NG].unsqueeze(1) \
                    .to_broadcast([P, n, NG])
                nc.gpsimd.tensor_mul(
                    out=tmp[:].rearrange("p (n g) -> p n g", g=NG),
                    in0=esb[:].rearrange("p (n g) -> p n g", g=NG),
                    in1=cqb)
                nc.vector.reduce_sum(
                    wq[:, q * n:(q + 1) * n],
                    tmp[:].rearrange("p (n g) -> p n g", g=NG),
                    axis=mybir.AxisListType.X)
                nc.vector.tensor_mul(
                    out=wq[:, q * n:(q + 1) * n],
                    in0=wq[:, q * n:(q + 1) * n],
                    in1=rsb[:])

            # --- main loop: gather + affine combine -----------------------
            def compute_chunk(t, av_slice, bv_slice):
                w0 = wq[:, 0 * n + t:0 * n + t + 1]
                w1 = wq[:, 1 * n + t:1 * n + t + 1]
                w2 = wq[:, 2 * n + t:2 * n + t + 1]
                w3 = wq[:, 3 * n + t:3 * n + t + 1]
                t1 = wpool.tile([P, batch], f32, tag="t1")
                t2 = wpool.tile([P, batch], f32, tag="t2")
                ot = wpool.tile([P, batch], f32, tag="ot")
                # t1 = w3*av + w2 ; t2 = w1*av + w0   (ACT engine)
                nc.scalar.activation(
                    t1[:], av_slice, mybir.ActivationFunctionType.Identity,
                    bias=w2, scale=w3)
                nc.scalar.activation(
                    t2[:], av_slice, mybir.ActivationFunctionType.Identity,
                    bias=w0, scale=w1)
                # ot = t1*bv + t2                     (DVE engine)
                nc.vector.tensor_mul(out=t1[:], in0=t1[:], in1=bv_slice)
                nc.vector.tensor_add(out=ot[:], in0=t1[:], in1=t2[:])
                eng = nc.sync if t % 2 == 0 else nc.scalar
                eng.dma_start(out=out[t * P:(t + 1) * P, :], in_=ot[:])

            t0 = 0
            for ha, hb, sg in sections:
                xt_a = xt[ha * half:(ha + 1) * half, :]
                xt_b = xt[hb * half:(hb + 1) * half, :]
                for s0 in range(0, sg, SUBS):
                    ns = min(SUBS, sg - s0)
                    rows = ns * P
                    c0 = (t0 + s0) * 8        # idx column = position // 16
                    av = gpool.tile([P, SUBS * batch], XT_DT, tag="av")
                    bv = gpool.tile([P, SUBS * batch], XT_DT, tag="bv")
                    nc.gpsimd.dma_gather(
                        av[:, :ns * batch].rearrange("p (s e) -> p s e", e=batch),
                        xt_a, iasb[:, c0:c0 + ns * 8], rows, rows, batch,
                        elem_step=batch, queue_num=0)
                    nc.gpsimd.dma_gather(
                        bv[:, :ns * batch].rearrange("p (s e) -> p s e", e=batch),
                        xt_b, ibsb[:, c0:c0 + ns * 8], rows, rows, batch,
                        elem_step=batch, queue_num=1)
                    for s in range(ns):
                        compute_chunk(t0 + s0 + s,
                                      av[:, s * batch:(s + 1) * batch],
                                      bv[:, s * batch:(s + 1) * batch])
                t0 += sg

    nc.compile()
    return nc


def shard_inputs(x, weights, a, b, in_dim=IN_DIM, n_cores=N_CORES, half=None):
    """Host-side layout prep.

    Returns (in_maps, perms, sections):
      in_maps[c]: tensors for core c
      perms[c]:   [n*128] global neuron id per slot (-1 = padding)
      sections:   [(ha, hb, n_chunks_g)] * 4, shared by all cores
    """
    if half is None:
        half = in_dim // 2
    x = np.asarray(x, dtype=np.float32)
    xt = np.ascontiguousarray(x.T.astype(XT_DT_NP))
    a = np.ascontiguousarray(np.asarray(a)).astype(np.int64)
    b = np.ascontiguousarray(np.asarray(b)).astype(np.int64)
    weights = np.asarray(weights, dtype=np.float32)
    out_dim = a.shape[0]

    group = (a >= half).astype(np.int64) * 2 + (b >= half).astype(np.int64)
    # Round-robin each group's neurons across cores -> per-core group lists
    # differ in size by at most 1, so a shared chunk count works.
    core_lists = [[[] for _ in range(4)] for _ in range(n_cores)]
    for g in range(4):
        ids = np.where(group == g)[0]
        for c in range(n_cores):
            core_lists[c][g] = ids[c::n_cores]
    sections = []
    for g in range(4):
        mx = max(len(core_lists[c][g]) for c in range(n_cores))
        sections.append((g >> 1, g & 1, (mx + P - 1) // P))
    n = sum(s for _, _, s in sections)

    in_maps, perms = [], []
    for c in range(n_cores):
        perm = np.full(n * P, -1, np.int64)
        t0 = 0
        for g in range(4):
            ids = core_lists[c][g]
            perm[t0 * P: t0 * P + len(ids)] = ids
            t0 += sections[g][2]
        valid = perm >= 0
        # int16 indices relative to each neuron's table half (0 for padding)
        rel_a = np.zeros(n * P, np.int16)
        rel_b = np.zeros(n * P, np.int16)
        rel_a[valid] = (a[perm[valid]] % half).astype(np.int16)
        rel_b[valid] = (b[perm[valid]] % half).astype(np.int16)

        def idx16(v):  # position i -> [i%16 (+16k replicas), i//16]
            w = v.reshape(n * 8, 16).T      # [16, n*8]
            return np.ascontiguousarray(np.tile(w, (8, 1)))

        wc = np.zeros((n * P, NG), np.float32)
        wc[valid] = weights[perm[valid]]
        wc = np.ascontiguousarray(
            wc.reshape(n, P, NG).transpose(1, 0, 2).reshape(P, n * NG))
        in_maps.append({
            "xt": xt,
            "ia": idx16(rel_a),
            "ib": idx16(rel_b),
            "wg": wc,
        })
        perms.append(perm)
    return in_maps, perms, sections


_CACHE = {}
LAST_RESULTS = None  # BassKernelResults of the most recent run (for profiling)


def kernel(x, weights, a, b):
    global LAST_RESULTS
    in_maps, perms, sections = shard_inputs(x, weights, a, b)
    key = tuple(sections)
    if key not in _CACHE:
        _CACHE[key] = build_program(sections)
    nc = _CACHE[key]

    res = run_bass_kernel_spmd(nc, in_maps, core_ids=list(range(N_CORES)))
    LAST_RESULTS = res

    outt = np.empty((OUT_DIM, BATCH), np.float32)
    for c, r in enumerate(res.results):
        perm = perms[c]
        valid = perm >= 0
        outt[perm[valid]] = r["out"][valid]
    return np.ascontiguousarray(outt.T)


# revision 14
# speedup vs baseline: 1.4583x; 1.4583x over previous
"""Trainium2 Bass kernel for the soft-logic-gate network (nn_Logic).

Math: for each output neuron o with wired inputs a[o], b[o]:
    av = x[:, a[o]], bv = x[:, b[o]], nw = softmax(weights[o])   (16 gates)
    r[:, o] = sum_g nw[g] * gate_g(av, bv)

Every one of the 16 soft gates is affine in (1, av, bv, av*bv), so the
mixture collapses to
    r = w0 + w1*av + w2*bv + w3*av*bv,   [w0..w3] = softmax(weights) @ C
with C a constant [16, 4] matrix. The kernel computes softmax+C on-chip,
gathers av/bv rows from a transposed fp16 x table with the GPSIMD dma_gather
instruction, and does the affine combine with 2 ACT + 2 DVE ops per
128-neuron chunk.

dma_gather takes int16 row indices, so the 65536-row table is treated as two
32768-row halves and neurons are grouped by which half (a, b) each fall in;
groups are padded to 128-row chunks and balanced round-robin across the 8
cores (out_dim sharding). Gathered row j lands at partition j%128,
slot j//128 — the whole layout (indices, weights, output rows) uses that
slot order, and the host undoes the permutation at the end.
"""

import numpy as np

import concourse.bacc as bacc
import concourse.mybir as mybir
import concourse.tile as tile
from concourse.bass_utils import run_bass_kernel_spmd

N_CORES = 8
IN_DIM = 65536
OUT_DIM = 65536
BATCH = 1024
NG = 16
P = 128
HALF = IN_DIM // 2          # dma_gather int16 index limit is 32767
SUBS = 4                    # slots (128-row groups) per dma_gather

# Gate g = c0 + c1*av + c2*bv + c3*av*bv, rows in module gate order 0..15.
GATE_COEF = np.array(
    [
        [0, 0, 0, 0],    # FALSE
        [0, 0, 0, 1],    # AND
        [0, 1, 0, -1],   # A AND NOT B
        [0, 1, 0, 0],    # A
        [0, 0, 1, -1],   # NOT A AND B
        [0, 0, 1, 0],    # B
        [0, 1, 1, -2],   # XOR
        [0, 1, 1, -1],   # OR
        [1, -1, -1, 1],  # NOR
        [1, -1, -1, 2],  # XNOR
        [1, 0, -1, 0],   # NOT B
        [1, 0, -1, 1],   # A OR NOT B
        [1, -1, 0, 0],   # NOT A
        [1, -1, 0, 1],   # NOT A OR B
        [1, 0, 0, -1],   # NAND
        [1, 0, 0, 0],    # TRUE
    ],
    dtype=np.float32,
)

f32 = mybir.dt.float32
f16 = mybir.dt.float16
i16 = mybir.dt.int16

# Gather-table dtype: float16 halves gather traffic (x is uniform [0,1), so
# fp16 quantization adds ~5e-4 relative error).
XT_DT = f16
XT_DT_NP = np.float16


def build_program(sections, in_dim=IN_DIM, batch=BATCH, half=None, bufs=3):
    """Trace the per-core Bass/Tile program (identical on all cores).

    sections: list of (ha, hb, n_chunks_g) — per neuron-group table halves
    and chunk counts (chunks of 128 neurons), shared by all cores.
    """
    if half is None:
        half = in_dim // 2
    n = sum(s for _, _, s in sections)    # total chunks (= slots/partition)

    nc = bacc.Bacc("TRN2", target_bir_lowering=False, debug=False,
                   enable_asserts=False, num_swdge_queues=2)

    xt = nc.dram_tensor("xt", [in_dim, batch], XT_DT, kind="ExternalInput")
    ia = nc.dram_tensor("ia", [P, n * 8], i16, kind="ExternalInput")
    ib = nc.dram_tensor("ib", [P, n * 8], i16, kind="ExternalInput")
    wg = nc.dram_tensor("wg", [P, n * NG], f32, kind="ExternalInput")
    out = nc.dram_tensor("out", [n * P, batch], f16, kind="ExternalOutput")

    # [128, 4*16] constant: row-replicated C^T (q-major, g-minor).
    cmat_np = np.tile(GATE_COEF.T.reshape(1, 4 * NG), (P, 1)).astype(np.float32)
    cm = nc.inline_tensor(cmat_np, name="cmat")

    with tile.TileContext(nc) as tc:
        with (
            tc.tile_pool(name="const", bufs=1) as cpool,
            tc.tile_pool(name="gath", bufs=bufs) as gpool,
            tc.tile_pool(name="work", bufs=2 * bufs) as wpool,
        ):
            # --- constant loads -------------------------------------------
            # indices: position i (slot order) lives at [i%16 (+16k), i//16].
            iasb = cpool.tile([P, n * 8], i16)
            ibsb = cpool.tile([P, n * 8], i16)
            nc.sync.dma_start(out=iasb[:], in_=ia[:])
            nc.sync.dma_start(out=ibsb[:], in_=ib[:])
            csb = cpool.tile([P, 4 * NG], f32)
            nc.sync.dma_start(out=csb[:], in_=cm.ap())
            wsb = cpool.tile([P, n * NG], f32)
            nc.sync.dma_start(out=wsb[:], in_=wg[:])

            # --- softmax(weights) @ C  ->  4 coefficient planes -----------
            # weights layout in SBUF: [p, (t, g)] with t = chunk slot
            # (neuron position = t*128 + p). randn weights are small, so exp
            # without max-subtraction is safe in f32.
            esb = cpool.tile([P, n * NG], f32)
            nc.scalar.activation(esb[:], wsb[:], mybir.ActivationFunctionType.Exp)
            ssb = cpool.tile([P, n], f32)
            nc.vector.reduce_sum(
                ssb[:], esb[:].rearrange("p (n g) -> p n g", g=NG),
                axis=mybir.AxisListType.X)
            rsb = cpool.tile([P, n], f32)
            nc.vector.reciprocal(rsb[:], ssb[:])

            wq = cpool.tile([P, 4 * n], f32)   # [p, (q, t)]
            tmp = cpool.tile([P, n * NG], f32)
            for q in range(4):
                cqb = csb[:, q * NG:(q + 1) * NG].unsqueeze(1) \
                    .to_broadcast([P, n, NG])
                nc.vector.tensor_mul(
                    out=tmp[:].rearrange("p (n g) -> p n g", g=NG),
                    in0=esb[:].rearrange("p (n g) -> p n g", g=NG),
                    in1=cqb)
                nc.vector.reduce_sum(
                    wq[:, q * n:(q + 1) * n],
                    tmp[:].rearrange("p (n g) -> p n g", g=NG),
                    axis=mybir.AxisListType.X)
                nc.vector.tensor_mul(
                    out=wq[:, q * n:(q + 1) * n],
                    in0=wq[:, q * n:(q + 1) * n],
                    in1=rsb[:])

            # --- main loop: gather + affine combine -----------------------
            def compute_chunk(t, av_slice, bv_slice):
                w0 = wq[:, 0 * n + t:0 * n + t + 1]
                w1 = wq[:, 1 * n + t:1 * n + t + 1]
                w2 = wq[:, 2 * n + t:2 * n + t + 1]
                w3 = wq[:, 3 * n + t:3 * n + t + 1]
                t1 = wpool.tile([P, batch], f32, tag="t1")
                t2 = wpool.tile([P, batch], f32, tag="t2")
                ot = wpool.tile([P, batch], f16, tag="ot")
                # t1 = w3*av + w2 ; t2 = w1*av + w0   (ACT engine)
                nc.scalar.activation(
                    t1[:], av_slice, mybir.ActivationFunctionType.Identity,
                    bias=w2, scale=w3)
                nc.scalar.activation(
                    t2[:], av_slice, mybir.ActivationFunctionType.Identity,
                    bias=w0, scale=w1)
                # ot = t1*bv + t2                     (DVE engine)
                nc.vector.tensor_mul(out=t1[:], in0=t1[:], in1=bv_slice)
                nc.vector.tensor_add(out=ot[:], in0=t1[:], in1=t2[:])
                nc.sync.dma_start(out=out[t * P:(t + 1) * P, :], in_=ot[:])

            t0 = 0
            for ha, hb, sg in sections:
                xt_a = xt[ha * half:(ha + 1) * half, :]
                xt_b = xt[hb * half:(hb + 1) * half, :]
                for s0 in range(0, sg, SUBS):
                    ns = min(SUBS, sg - s0)
                    rows = ns * P
                    c0 = (t0 + s0) * 8        # idx column = position // 16
                    av = gpool.tile([P, SUBS * batch], XT_DT, tag="av")
                    bv = gpool.tile([P, SUBS * batch], XT_DT, tag="bv")
                    nc.gpsimd.dma_gather(
                        av[:, :ns * batch].rearrange("p (s e) -> p s e", e=batch),
                        xt_a, iasb[:, c0:c0 + ns * 8], rows, rows, batch,
                        elem_step=batch, queue_num=0)
                    nc.gpsimd.dma_gather(
                        bv[:, :ns * batch].rearrange("p (s e) -> p s e", e=batch),
                        xt_b, ibsb[:, c0:c0 + ns * 8], rows, rows, batch,
                        elem_step=batch, queue_num=1)
                    for s in range(ns):
                        compute_chunk(t0 + s0 + s,
                                      av[:, s * batch:(s + 1) * batch],
                                      bv[:, s * batch:(s + 1) * batch])
                t0 += sg

    nc.compile()
    return nc


def shard_inputs(x, weights, a, b, in_dim=IN_DIM, n_cores=N_CORES, half=None):
    """Host-side layout prep.

    Returns (in_maps, perms, sections):
      in_maps[c]: tensors for core c
      perms[c]:   [n*128] global neuron id per slot (-1 = padding)
      sections:   [(ha, hb, n_chunks_g)] * 4, shared by all cores
    """
    if half is None:
        half = in_dim // 2
    x = np.asarray(x, dtype=np.float32)
    xt = np.ascontiguousarray(x.T.astype(XT_DT_NP))
    a = np.ascontiguousarray(np.asarray(a)).astype(np.int64)
    b = np.ascontiguousarray(np.asarray(b)).astype(np.int64)
    weights = np.asarray(weights, dtype=np.float32)
    out_dim = a.shape[0]

    group = (a >= half).astype(np.int64) * 2 + (b >= half).astype(np.int64)
    # Spread each group's neurons across cores as evenly as possible; the
    # shared per-section chunk count is the global minimum ceil(N_g/(128*8)).
    core_lists = [[None] * 4 for _ in range(n_cores)]
    sections = []
    for g in range(4):
        ids = np.where(group == g)[0]
        counts = [len(ids) // n_cores + (1 if c < len(ids) % n_cores else 0)
                  for c in range(n_cores)]
        off = 0
        for c in range(n_cores):
            core_lists[c][g] = ids[off:off + counts[c]]
            off += counts[c]
        sections.append((g >> 1, g & 1, (max(counts) + P - 1) // P))
    n = sum(s for _, _, s in sections)

    in_maps, perms = [], []
    for c in range(n_cores):
        perm = np.full(n * P, -1, np.int64)
        t0 = 0
        for g in range(4):
            ids = core_lists[c][g]
            perm[t0 * P: t0 * P + len(ids)] = ids
            t0 += sections[g][2]
        valid = perm >= 0
        # int16 indices relative to each neuron's table half (0 for padding)
        rel_a = np.zeros(n * P, np.int16)
        rel_b = np.zeros(n * P, np.int16)
        rel_a[valid] = (a[perm[valid]] % half).astype(np.int16)
        rel_b[valid] = (b[perm[valid]] % half).astype(np.int16)

        def idx16(v):  # position i -> [i%16 (+16k replicas), i//16]
            w = v.reshape(n * 8, 16).T      # [16, n*8]
            return np.ascontiguousarray(np.tile(w, (8, 1)))

        wc = np.zeros((n * P, NG), np.float32)
        wc[valid] = weights[perm[valid]]
        wc = np.ascontiguousarray(
            wc.reshape(n, P, NG).transpose(1, 0, 2).reshape(P, n * NG))
        in_maps.append({
            "xt": xt,
            "ia": idx16(rel_a),
            "ib": idx16(rel_b),
            "wg": wc,
        })
        perms.append(perm)
    return in_maps, perms, sections


_CACHE = {}
LAST_RESULTS = None  # BassKernelResults of the most recent run (for profiling)


def kernel(x, weights, a, b):
    global LAST_RESULTS
    in_maps, perms, sections = shard_inputs(x, weights, a, b)
    key = tuple(sections)
    if key not in _CACHE:
        _CACHE[key] = build_program(sections)
    nc = _CACHE[key]

    res = run_bass_kernel_spmd(nc, in_maps, core_ids=list(range(N_CORES)))
    LAST_RESULTS = res

    outt = np.empty((OUT_DIM, BATCH), np.float32)
    for c, r in enumerate(res.results):
        perm = perms[c]
        valid = perm >= 0
        outt[perm[valid]] = r["out"][valid].astype(np.float32)
    return np.ascontiguousarray(outt.T)


# revision 15
# speedup vs baseline: 1.7904x; 1.2277x over previous
"""Trainium2 Bass kernel for the soft-logic-gate network (nn_Logic).

Math: for each output neuron o with wired inputs a[o], b[o]:
    av = x[:, a[o]], bv = x[:, b[o]], nw = softmax(weights[o])   (16 gates)
    r[:, o] = sum_g nw[g] * gate_g(av, bv)

Every one of the 16 soft gates is affine in (1, av, bv, av*bv), so the
mixture collapses to
    r = w0 + w1*av + w2*bv + w3*av*bv,   [w0..w3] = softmax(weights) @ C
with C a constant [16, 4] matrix. The kernel computes softmax+C on-chip,
gathers av/bv rows from a transposed fp16 x table with the GPSIMD dma_gather
instruction, and does the affine combine with 2 ACT + 2 DVE ops per
128-neuron chunk.

dma_gather takes int16 row indices, so the 65536-row table is treated as two
32768-row halves and neurons are grouped by which half (a, b) each fall in;
groups are padded to 128-row chunks and balanced round-robin across the 8
cores (out_dim sharding). Gathered row j lands at partition j%128,
slot j//128 — the whole layout (indices, weights, output rows) uses that
slot order, and the host undoes the permutation at the end.
"""

import numpy as np

import concourse.bacc as bacc
import concourse.mybir as mybir
import concourse.tile as tile
from concourse.bass_utils import run_bass_kernel_spmd

N_CORES = 8
IN_DIM = 65536
OUT_DIM = 65536
BATCH = 1024
NG = 16
P = 128
HALF = IN_DIM // 2          # dma_gather int16 index limit is 32767
SUBS = 4                    # slots (128-row groups) per dma_gather

# Gate g = c0 + c1*av + c2*bv + c3*av*bv, rows in module gate order 0..15.
GATE_COEF = np.array(
    [
        [0, 0, 0, 0],    # FALSE
        [0, 0, 0, 1],    # AND
        [0, 1, 0, -1],   # A AND NOT B
        [0, 1, 0, 0],    # A
        [0, 0, 1, -1],   # NOT A AND B
        [0, 0, 1, 0],    # B
        [0, 1, 1, -2],   # XOR
        [0, 1, 1, -1],   # OR
        [1, -1, -1, 1],  # NOR
        [1, -1, -1, 2],  # XNOR
        [1, 0, -1, 0],   # NOT B
        [1, 0, -1, 1],   # A OR NOT B
        [1, -1, 0, 0],   # NOT A
        [1, -1, 0, 1],   # NOT A OR B
        [1, 0, 0, -1],   # NAND
        [1, 0, 0, 0],    # TRUE
    ],
    dtype=np.float32,
)

f32 = mybir.dt.float32
f16 = mybir.dt.float16
i16 = mybir.dt.int16

# Gather-table dtype: float16 halves gather traffic (x is uniform [0,1), so
# fp16 quantization adds ~5e-4 relative error).
XT_DT = f16
XT_DT_NP = np.float16


def build_program(sections, in_dim=IN_DIM, batch=BATCH, half=None, bufs=3):
    """Trace the per-core Bass/Tile program (identical on all cores).

    sections: list of (ha, hb, n_chunks_g) — per neuron-group table halves
    and chunk counts (chunks of 128 neurons), shared by all cores.
    """
    if half is None:
        half = in_dim // 2
    n = sum(s for _, _, s in sections)    # total chunks (= slots/partition)

    nc = bacc.Bacc("TRN2", target_bir_lowering=False, debug=False,
                   enable_asserts=False, num_swdge_queues=2)

    xt = nc.dram_tensor("xt", [in_dim, batch], XT_DT, kind="ExternalInput")
    ia = nc.dram_tensor("ia", [P, n * 8], i16, kind="ExternalInput")
    ib = nc.dram_tensor("ib", [P, n * 8], i16, kind="ExternalInput")
    wg = nc.dram_tensor("wg", [P, n * NG], f32, kind="ExternalInput")
    out = nc.dram_tensor("out", [n * P, batch], f16, kind="ExternalOutput")

    # [128, 4*16] constant: row-replicated C^T (q-major, g-minor).
    cmat_np = np.tile(GATE_COEF.T.reshape(1, 4 * NG), (P, 1)).astype(np.float32)
    cm = nc.inline_tensor(cmat_np, name="cmat")

    with tile.TileContext(nc) as tc:
        with (
            tc.tile_pool(name="const", bufs=1) as cpool,
            tc.tile_pool(name="gath", bufs=bufs) as gpool,
            tc.tile_pool(name="work", bufs=2 * bufs) as wpool,
        ):
            # --- constant loads -------------------------------------------
            # indices: position i (slot order) lives at [i%16 (+16k), i//16].
            iasb = cpool.tile([P, n * 8], i16)
            ibsb = cpool.tile([P, n * 8], i16)
            nc.sync.dma_start(out=iasb[:], in_=ia[:])
            nc.sync.dma_start(out=ibsb[:], in_=ib[:])
            csb = cpool.tile([P, 4 * NG], f32)
            nc.sync.dma_start(out=csb[:], in_=cm.ap())
            wsb = cpool.tile([P, n * NG], f32)
            nc.sync.dma_start(out=wsb[:], in_=wg[:])

            # --- softmax(weights) @ C  ->  4 coefficient planes -----------
            # weights layout in SBUF: [p, (t, g)] with t = chunk slot
            # (neuron position = t*128 + p). randn weights are small, so exp
            # without max-subtraction is safe in f32.
            esb = cpool.tile([P, n * NG], f32)
            nc.scalar.activation(esb[:], wsb[:], mybir.ActivationFunctionType.Exp)
            ssb = cpool.tile([P, n], f32)
            nc.vector.reduce_sum(
                ssb[:], esb[:].rearrange("p (n g) -> p n g", g=NG),
                axis=mybir.AxisListType.X)
            rsb = cpool.tile([P, n], f32)
            nc.vector.reciprocal(rsb[:], ssb[:])

            wq = cpool.tile([P, 4 * n], f32)   # [p, (q, t)]
            tmp = cpool.tile([P, n * NG], f32)
            for q in range(4):
                cqb = csb[:, q * NG:(q + 1) * NG].unsqueeze(1) \
                    .to_broadcast([P, n, NG])
                nc.vector.tensor_mul(
                    out=tmp[:].rearrange("p (n g) -> p n g", g=NG),
                    in0=esb[:].rearrange("p (n g) -> p n g", g=NG),
                    in1=cqb)
                nc.vector.reduce_sum(
                    wq[:, q * n:(q + 1) * n],
                    tmp[:].rearrange("p (n g) -> p n g", g=NG),
                    axis=mybir.AxisListType.X)
                nc.vector.tensor_mul(
                    out=wq[:, q * n:(q + 1) * n],
                    in0=wq[:, q * n:(q + 1) * n],
                    in1=rsb[:])

            # --- main loop: gather + affine combine -----------------------
            def compute_chunk(t, av_slice, bv_slice):
                w0 = wq[:, 0 * n + t:0 * n + t + 1]
                w1 = wq[:, 1 * n + t:1 * n + t + 1]
                w2 = wq[:, 2 * n + t:2 * n + t + 1]
                w3 = wq[:, 3 * n + t:3 * n + t + 1]
                t1 = wpool.tile([P, batch], f16, tag="t1")
                t2 = wpool.tile([P, batch], f16, tag="t2")
                ot = wpool.tile([P, batch], f16, tag="ot")
                # t1 = w3*av + w2 ; t2 = w1*av + w0   (ACT engine)
                nc.scalar.activation(
                    t1[:], av_slice, mybir.ActivationFunctionType.Identity,
                    bias=w2, scale=w3)
                nc.scalar.activation(
                    t2[:], av_slice, mybir.ActivationFunctionType.Identity,
                    bias=w0, scale=w1)
                # ot = t1*bv + t2                     (DVE engine)
                nc.vector.tensor_mul(out=t1[:], in0=t1[:], in1=bv_slice)
                nc.vector.tensor_add(out=ot[:], in0=t1[:], in1=t2[:])
                nc.sync.dma_start(out=out[t * P:(t + 1) * P, :], in_=ot[:])

            t0 = 0
            for ha, hb, sg in sections:
                xt_a = xt[ha * half:(ha + 1) * half, :]
                xt_b = xt[hb * half:(hb + 1) * half, :]
                for s0 in range(0, sg, SUBS):
                    ns = min(SUBS, sg - s0)
                    rows = ns * P
                    c0 = (t0 + s0) * 8        # idx column = position // 16
                    av = gpool.tile([P, SUBS * batch], XT_DT, tag="av")
                    bv = gpool.tile([P, SUBS * batch], XT_DT, tag="bv")
                    nc.gpsimd.dma_gather(
                        av[:, :ns * batch].rearrange("p (s e) -> p s e", e=batch),
                        xt_a, iasb[:, c0:c0 + ns * 8], rows, rows, batch,
                        elem_step=batch, queue_num=0)
                    nc.gpsimd.dma_gather(
                        bv[:, :ns * batch].rearrange("p (s e) -> p s e", e=batch),
                        xt_b, ibsb[:, c0:c0 + ns * 8], rows, rows, batch,
                        elem_step=batch, queue_num=1)
                    for s in range(ns):
                        compute_chunk(t0 + s0 + s,
                                      av[:, s * batch:(s + 1) * batch],
                                      bv[:, s * batch:(s + 1) * batch])
                t0 += sg

    nc.compile()
    return nc


def shard_inputs(x, weights, a, b, in_dim=IN_DIM, n_cores=N_CORES, half=None):
    """Host-side layout prep.

    Returns (in_maps, perms, sections):
      in_maps[c]: tensors for core c
      perms[c]:   [n*128] global neuron id per slot (-1 = padding)
      sections:   [(ha, hb, n_chunks_g)] * 4, shared by all cores
    """
    if half is None:
        half = in_dim // 2
    x = np.asarray(x, dtype=np.float32)
    xt = np.ascontiguousarray(x.T.astype(XT_DT_NP))
    a = np.ascontiguousarray(np.asarray(a)).astype(np.int64)
    b = np.ascontiguousarray(np.asarray(b)).astype(np.int64)
    weights = np.asarray(weights, dtype=np.float32)
    out_dim = a.shape[0]

    group = (a >= half).astype(np.int64) * 2 + (b >= half).astype(np.int64)
    # Spread each group's neurons across cores as evenly as possible; the
    # shared per-section chunk count is the global minimum ceil(N_g/(128*8)).
    core_lists = [[None] * 4 for _ in range(n_cores)]
    sections = []
    for g in range(4):
        ids = np.where(group == g)[0]
        counts = [len(ids) // n_cores + (1 if c < len(ids) % n_cores else 0)
                  for c in range(n_cores)]
        off = 0
        for c in range(n_cores):
            core_lists[c][g] = ids[off:off + counts[c]]
            off += counts[c]
        sections.append((g >> 1, g & 1, (max(counts) + P - 1) // P))
    n = sum(s for _, _, s in sections)

    in_maps, perms = [], []
    for c in range(n_cores):
        perm = np.full(n * P, -1, np.int64)
        t0 = 0
        for g in range(4):
            ids = core_lists[c][g]
            perm[t0 * P: t0 * P + len(ids)] = ids
            t0 += sections[g][2]
        valid = perm >= 0
        # int16 indices relative to each neuron's table half (0 for padding)
        rel_a = np.zeros(n * P, np.int16)
        rel_b = np.zeros(n * P, np.int16)
        rel_a[valid] = (a[perm[valid]] % half).astype(np.int16)
        rel_b[valid] = (b[perm[valid]] % half).astype(np.int16)

        def idx16(v):  # position i -> [i%16 (+16k replicas), i//16]
            w = v.reshape(n * 8, 16).T      # [16, n*8]
            return np.ascontiguousarray(np.tile(w, (8, 1)))

        wc = np.zeros((n * P, NG), np.float32)
        wc[valid] = weights[perm[valid]]
        wc = np.ascontiguousarray(
            wc.reshape(n, P, NG).transpose(1, 0, 2).reshape(P, n * NG))
        in_maps.append({
            "xt": xt,
            "ia": idx16(rel_a),
            "ib": idx16(rel_b),
            "wg": wc,
        })
        perms.append(perm)
    return in_maps, perms, sections


_CACHE = {}
LAST_RESULTS = None  # BassKernelResults of the most recent run (for profiling)


def kernel(x, weights, a, b):
    global LAST_RESULTS
    in_maps, perms, sections = shard_inputs(x, weights, a, b)
    key = tuple(sections)
    if key not in _CACHE:
        _CACHE[key] = build_program(sections)
    nc = _CACHE[key]

    res = run_bass_kernel_spmd(nc, in_maps, core_ids=list(range(N_CORES)))
    LAST_RESULTS = res

    outt = np.empty((OUT_DIM, BATCH), np.float32)
    for c, r in enumerate(res.results):
        perm = perms[c]
        valid = perm >= 0
        outt[perm[valid]] = r["out"][valid].astype(np.float32)
    return np.ascontiguousarray(outt.T)
